# revision 28
# baseline (speedup 1.0000x reference)
"""Trainium2 Bass kernel for nn_ComplexMixture (weighted complex density
matrices).

Reference computation (B=4, S=8192, D=512):
    out_r[b] = sum_s w[b,s] * (r_s r_s^T + i_s i_s^T)   -> [B, D, D]
    out_i[b] = sum_s w[b,s] * (i_s r_s^T - r_s i_s^T)   -> [B, D, D]

Strategy (8 NeuronCores), v8:
  - Shard (b, S-half): core k handles batch k//2, S rows [4096*(k%2), +4096).
  - Host precomputes (bf16, partition-major [128, T, D] layout):
        Us = 0.5*sqrt(w)*(R+I),   Vs = 0.5*sqrt(w)*(R-I)
    With H = Us^T Vs (full, bf16) and P = Rs^T Rs (upper triangle),
    Rs = Us+Vs = sqrt(w)*R (one DVE add per chunk, fp8-e4m3 output):
        out_i = 2*(H - H^T)               (bf16 accuracy, ~2.3e-3)
        out_r = 2*P_full - 2*(H + H^T)    (fp8 P, ~1.1e-2; gate is 2e-2)
  - P row-blocks 0..2 use fp8 DoubleRow matmuls (contract 2 s-tiles per
    instruction, ~2x ALU); P3 (128 cols) stays plain fp8 (DoubleRow loses
    below ~256 free dim because it disables fast weight load).
  - Phase A (tile-pair, bank-major in pair): H0..H3 (bf16, 512 cols) +
    P0,P1 DoubleRow. Phase B: P2 DoubleRow, then P3 plain. Earlier banks
    flush under phase B; tail is only P3's flush on the ACT ring.
  - 8 PSUM banks: ph0-3 (H row blocks) + pp0-3 (P upper rows).
"""

import sys

if "/opt/trn_rl_repo" not in sys.path:
    sys.path.insert(0, "/opt/trn_rl_repo")

import numpy as np

B, S, D = 4, 8192, 512
N_CORES = 8
S_LOC = S // 2          # rows per core
P = 128                 # SBUF partitions
T = S_LOC // P          # 32 s-tiles per core
M = D // P              # 4 row-blocks of the DxD outputs

# DMA chunk sizes in s-tiles after the 4-ring lead-in (tiles 0 and 1 of
# each tensor issue on their own rings): fine enough that early
# tile-pairs are not stuck behind big transfers.
CHUNKS = ((2, 2), (4, 2), (6, 2), (8, 4), (12, 4),
          (16, 4), (20, 4), (24, 4), (28, 4))

_cache = {}


def _split_multi_waits(bir: bytes) -> bytes:
    """This container's walrus build accepts at most one sync-wait command
    per instruction ("Too many sync wait commands"), while Tile freely packs
    several. Splitting the extras into preceding single-wait NoOps on the
    same engine is semantically identical for monotonic sem-ge waits: the
    sequencer blocks on each in turn before dispatching the instruction.
    """
    import json

    m = json.loads(bir)
    n = [0]

    def fix(obj):
        if isinstance(obj, dict):
            insts = obj.get("instructions")
            if isinstance(insts, list) and insts and isinstance(insts[0], dict):
                out = []
                for inst in insts:
                    si = inst.get("sync_info")
                    waits = (si or {}).get("on_wait") or []
                    cap = 2 if inst.get("opcode") == "EventSemaphore" else 1
                    if len(waits) > cap and all(
                        w.get("wait_mode") == "sem-ge-imm" for w in waits
                    ):
                        for w in waits[:-cap]:
                            n[0] += 1
                            nop = {
                                "engine": inst["engine"],
                                "ins": [],
                                "name": f"{inst['name']}-ws{n[0]}",
                                "opcode": "NoOp",
                                "outs": [],
                                "sync_info": {"on_wait": [w], "on_update": []},
                                "text_hint": "wait_split",
                            }
                            if "debug" in inst:
                                nop["debug"] = inst["debug"]
                            out.append(nop)
                        si["on_wait"] = waits[-cap:]
                    out.append(inst)
                obj["instructions"] = out
            for v in obj.values():
                fix(v)
        elif isinstance(obj, list):
            for v in obj:
                fix(v)

    fix(m)
    return json.dumps(m).encode()


def _install_wait_split_patch(bass):
    if getattr(bass.Bass, "_wait_split_patched", False):
        return
    orig = bass.Bass.to_json_bytes

    def to_json_bytes(self, *a, **kw):
        return _split_multi_waits(orig(self, *a, **kw))

    bass.Bass.to_json_bytes = to_json_bytes
    bass.Bass._wait_split_patched = True


def _build():
    import concourse.bass as bass
    import concourse.tile as tile
    from concourse import mybir

    _install_wait_split_patch(bass)
    f32 = mybir.dt.float32
    bf16 = mybir.dt.bfloat16
    f8 = mybir.dt.float8e4
    DR = mybir.MatmulPerfMode.DoubleRow

    nc = bass.Bass(enable_partition_id=False)
    xu = nc.dram_tensor("xu", [P, T, D], bf16, kind="ExternalInput")
    xv = nc.dram_tensor("xv", [P, T, D], bf16, kind="ExternalInput")
    out_h = nc.dram_tensor("out_h", [D, D], f32, kind="ExternalOutput")
    out_p = nc.dram_tensor("out_p", [D, D], f32, kind="ExternalOutput")

    with tile.TileContext(nc) as tc:
        with (
            tc.tile_pool(name="big", bufs=1) as big,
            tc.tile_pool(name="wp", bufs=1) as wp,
            tc.tile_pool(name="psum", bufs=1, space="PSUM") as psum,
            tc.tile_pool(name="ost", bufs=8) as ost,
        ):
            us = big.tile([P, T, D], bf16, name="us", tag="us")
            vs = big.tile([P, T, D], bf16, name="vs", tag="vs")
            rs8 = big.tile([P, T, D], f8, name="rs8", tag="rs8")
            dmy = wp.tile([P, D], bf16, name="dmy", tag="dmy")
            dmf = wp.tile([P, 1], f32, name="dmf", tag="dmf")

            # Warm-up fodder init on the otherwise-idle Pool engine so the
            # PE dummies depend on nothing slow.
            nc.gpsimd.memset(dmy[:], 0.0)
            nc.gpsimd.memset(dmf[:], 0.0)

            ph = [psum.tile([P, D], f32, name=f"ph{m}", tag=f"ph{m}") for m in range(M)]
            pp = [psum.tile([P, D], f32, name=f"pp{m}", tag=f"pp{m}") for m in range(M)]

            # PE warm-up: HAM un-throttles after ~3.4us of sustained matmul
            # activity. Isolated N=512 dummies (~1.2us each cold) bridge
            # until the first input chunk lands (~11us). Dummies go to pp3's
            # bank (its first real start=True matmul discards this).
            for _ in range(3):
                nc.tensor.matmul(pp[3][:], dmy[:, :P], dmy[:], start=True, stop=True)

            # ---- input streaming + Rs8 = fp8(Us + Vs) ----------------
            # Lead-in: tiles 0 and 1 issue on three HWDGE rings in
            # parallel (DVE has no DMA ring) so the first pair lands ~1us
            # sooner (the fixed per-DMA pipeline latency dominates the
            # lead).
            nc.sync.dma_start(us[:, 0:1, :], xu[:, 0:1, :])
            nc.scalar.dma_start(vs[:, 0:1, :], xv[:, 0:1, :])
            nc.gpsimd.dma_start(us[:, 1:2, :], xu[:, 1:2, :])
            nc.scalar.dma_start(vs[:, 1:2, :], xv[:, 1:2, :])
            nc.vector.tensor_add(rs8[:, 0:2, :], us[:, 0:2, :], vs[:, 0:2, :])
            # Steady stream: xu on the SP ring, xv on the ACT ring; one
            # DVE add per chunk with fp8 output.
            for t0, nt in CHUNKS:
                sl = slice(t0, t0 + nt)
                nc.sync.dma_start(us[:, sl, :], xu[:, sl, :])
                nc.scalar.dma_start(vs[:, sl, :], xv[:, sl, :])
                nc.vector.tensor_add(rs8[:, sl, :], us[:, sl, :], vs[:, sl, :])

            # ACT Copy-table preload: issued after the xv loads so the
            # ~1.3us table load happens off the critical path, well before
            # the first PSUM flush copy needs it.
            nc.scalar.mul(dmf[:], dmf[:], 1.0)

            # ---- Phase A matmuls: tile-pair, bank-major within pair ---
            # H0..3: bf16, 512 cols, two tiles back-to-back per bank.
            # P0, P1: one fp8 DoubleRow matmul per pair (contracts both
            # tiles at once).
            # The first two pairs also run H3 (tiles 0-3): extra PE work
            # while the DMA pipeline fills, so the PE never goes idle (an
            # idle PE resets the HAM ramp window). H3's accumulation group
            # stays open; the H3 pass below finishes tiles 4..31.
            H3_HEAD = 4
            for tp in range(0, T, 2):
                st, sp = (tp == 0), (tp == T - 2)
                nbank = 4 if tp < H3_HEAD else 3
                # Pair 0 runs t0-major: four matmuls of tile-0 work buffer
                # the slightly later tile-1 DMA arrival.
                if tp == 0:
                    order = [(m, t) for t in (0, 1) for m in range(nbank)]
                else:
                    order = [(m, t) for m in range(nbank) for t in (tp, tp + 1)]
                for m, t in order:
                    nc.tensor.matmul(
                        ph[m][:],
                        us[:, t, m * P : (m + 1) * P],
                        vs[:, t, :],
                        start=(t == 0), stop=(t == T - 1),
                    )
                for m in range(2):
                    nc.tensor.matmul(
                        pp[m][:, m * P : D],
                        rs8[:, tp : tp + 2, m * P : (m + 1) * P],
                        rs8[:, tp : tp + 2, m * P : D],
                        start=st, stop=sp,
                        perf_mode=DR,
                    )

            # Flush the five phase-A banks (H0-2, P0, P1) while the H3
            # pass runs on the PE — their ~1.4 MiB of output DMA hides
            # under the remaining compute. Stores on the idle Pool ring so
            # the ACT queue stays clear for pp3's tail flush.
            hstage = ost.tile([P, M, D], f32, name="hstage", tag="hstage")
            out_h3 = out_h.rearrange("(m p) d -> p m d", p=P)
            nc.vector.tensor_copy(hstage[:, 0, :], ph[0][:])
            nc.scalar.copy(hstage[:, 1, :], ph[1][:])
            nc.vector.tensor_copy(hstage[:, 2, :], ph[2][:])
            nc.gpsimd.dma_start(out_h3[:, 0:3, :], hstage[:, 0:3, :])
            for m in range(2):
                c0 = m * P
                o = ost.tile([P, D - c0], f32, name=f"op{m}", tag="ostp")
                if m == 0:
                    nc.vector.tensor_copy(o[:], pp[m][:, c0:D])
                else:
                    nc.scalar.copy(o[:], pp[m][:, c0:D])
                nc.gpsimd.dma_start(out_p[m * P : (m + 1) * P, c0:D], o[:])

            # ---- H3 pass (SBUF-resident, same bank throughout) --------
            for t in range(H3_HEAD, T):
                nc.tensor.matmul(
                    ph[3][:],
                    us[:, t, 3 * P : D],
                    vs[:, t, :],
                    start=(t == 0), stop=(t == T - 1),
                )
            nc.scalar.copy(hstage[:, 3, :], ph[3][:])
            nc.gpsimd.dma_start(out_h3[:, 3:4, :], hstage[:, 3:4, :])

            # ---- Phase B: pp2 (DoubleRow pairs), pp3 (plain fp8) ------
            # pp3 (the last bank) flushes copy+store on one engine (ACT)
            # to keep the tail short.
            for tp in range(0, T, 2):
                nc.tensor.matmul(
                    pp[2][:, 2 * P : D],
                    rs8[:, tp : tp + 2, 2 * P : 3 * P],
                    rs8[:, tp : tp + 2, 2 * P : D],
                    start=(tp == 0), stop=(tp == T - 2),
                    perf_mode=DR,
                )
            o2 = ost.tile([P, D - 2 * P], f32, name="op2", tag="ostp")
            nc.vector.tensor_copy(o2[:], pp[2][:, 2 * P : D])
            nc.gpsimd.dma_start(out_p[2 * P : 3 * P, 2 * P : D], o2[:])

            for t in range(T):
                nc.tensor.matmul(
                    pp[3][:, 3 * P : D],
                    rs8[:, t, 3 * P : D],
                    rs8[:, t, 3 * P : D],
                    start=(t == 0), stop=(t == T - 1),
                )
            o3 = ost.tile([P, D - 3 * P], f32, name="op3", tag="ostp")
            nc.scalar.copy(o3[:], pp[3][:, 3 * P : D])
            nc.scalar.dma_start(out_p[3 * P : D, 3 * P : D], o3[:])

    return nc


def _get_nc():
    if "nc" not in _cache:
        _cache["nc"] = _build()
    return _cache["nc"]


def kernel(input_real, input_imag, weight):
    import ml_dtypes

    from concourse.bass_utils import run_bass_kernel_spmd

    bf16 = ml_dtypes.bfloat16
    input_real = np.asarray(input_real, dtype=np.float32)
    input_imag = np.asarray(input_imag, dtype=np.float32)
    weight = np.asarray(weight, dtype=np.float32)
    a = 0.5 * np.sqrt(weight)  # w >= 0 (uniform fill)

    us_full = (a[:, :, None] * (input_real + input_imag)).astype(bf16)
    vs_full = (a[:, :, None] * (input_real - input_imag)).astype(bf16)

    def pmaj(x):  # [S_LOC, D] -> [P, T, D], s_local = t*P + p
        return np.ascontiguousarray(x.reshape(T, P, D).transpose(1, 0, 2))

    in_maps = []
    for k in range(N_CORES):
        b, h = k // 2, k % 2
        rows = slice(h * S_LOC, (h + 1) * S_LOC)
        in_maps.append({"xu": pmaj(us_full[b, rows]), "xv": pmaj(vs_full[b, rows])})

    res = run_bass_kernel_spmd(
        _get_nc(), in_maps, core_ids=list(range(N_CORES))
    )

    out_r = np.empty((B, D, D), dtype=np.float32)
    out_i = np.empty((B, D, D), dtype=np.float32)
    for b in range(B):
        H = res.results[2 * b]["out_h"].astype(np.float64) + res.results[
            2 * b + 1
        ]["out_h"].astype(np.float64)
        Pu = res.results[2 * b]["out_p"].astype(np.float64) + res.results[
            2 * b + 1
        ]["out_p"].astype(np.float64)
        Pf = np.empty((D, D), dtype=np.float64)
        for m in range(M):
            for n in range(M):
                rm = slice(m * P, (m + 1) * P)
                rn = slice(n * P, (n + 1) * P)
                if m <= n:
                    Pf[rm, rn] = Pu[rm, rn]
                else:
                    Pf[rm, rn] = Pu[rn, rm].T
        Hs = H + H.T
        out_r[b] = (2.0 * Pf - 2.0 * Hs).astype(np.float32)
        out_i[b] = (2.0 * (H - H.T)).astype(np.float32)
    return out_r, out_i


# revision 30
# speedup vs baseline: 1.0324x; 1.0324x over previous
"""Trainium2 Bass kernel for nn_ComplexMixture (weighted complex density
matrices).

Reference computation (B=4, S=8192, D=512):
    out_r[b] = sum_s w[b,s] * (r_s r_s^T + i_s i_s^T)   -> [B, D, D]
    out_i[b] = sum_s w[b,s] * (i_s r_s^T - r_s i_s^T)   -> [B, D, D]

Strategy (8 NeuronCores), v8:
  - Shard (b, S-half): core k handles batch k//2, S rows [4096*(k%2), +4096).
  - Host precomputes (bf16, partition-major [128, T, D] layout):
        Us = 0.5*sqrt(w)*(R+I),   Vs = 0.5*sqrt(w)*(R-I)
    With H = Us^T Vs (full, bf16) and P = Rs^T Rs (upper triangle),
    Rs = Us+Vs = sqrt(w)*R (one DVE add per chunk, fp8-e4m3 output):
        out_i = 2*(H - H^T)               (bf16 accuracy, ~2.3e-3)
        out_r = 2*P_full - 2*(H + H^T)    (fp8 P, ~1.1e-2; gate is 2e-2)
  - P row-blocks 0..2 use fp8 DoubleRow matmuls (contract 2 s-tiles per
    instruction, ~2x ALU); P3 (128 cols) stays plain fp8 (DoubleRow loses
    below ~256 free dim because it disables fast weight load).
  - Phase A (tile-pair, bank-major in pair): H0..H3 (bf16, 512 cols) +
    P0,P1 DoubleRow. Phase B: P2 DoubleRow, then P3 plain. Earlier banks
    flush under phase B; tail is only P3's flush on the ACT ring.
  - 8 PSUM banks: ph0-3 (H row blocks) + pp0-3 (P upper rows).
"""

import sys

if "/opt/trn_rl_repo" not in sys.path:
    sys.path.insert(0, "/opt/trn_rl_repo")

import numpy as np

B, S, D = 4, 8192, 512
N_CORES = 8
S_LOC = S // 2          # rows per core
P = 128                 # SBUF partitions
T = S_LOC // P          # 32 s-tiles per core
M = D // P              # 4 row-blocks of the DxD outputs

# DMA chunk sizes in s-tiles: small lead-in for a fast PE start, fine
# enough that early tile-pairs are not stuck behind big transfers.
CHUNKS = ((0, 1), (1, 1), (2, 2), (4, 2), (6, 2), (8, 4), (12, 4),
          (16, 4), (20, 4), (24, 4), (28, 4))

_cache = {}


def _split_multi_waits(bir: bytes) -> bytes:
    """This container's walrus build accepts at most one sync-wait command
    per instruction ("Too many sync wait commands"), while Tile freely packs
    several. Splitting the extras into preceding single-wait NoOps on the
    same engine is semantically identical for monotonic sem-ge waits: the
    sequencer blocks on each in turn before dispatching the instruction.
    """
    import json

    m = json.loads(bir)
    n = [0]

    def fix(obj):
        if isinstance(obj, dict):
            insts = obj.get("instructions")
            if isinstance(insts, list) and insts and isinstance(insts[0], dict):
                out = []
                for inst in insts:
                    si = inst.get("sync_info")
                    waits = (si or {}).get("on_wait") or []
                    cap = 2 if inst.get("opcode") == "EventSemaphore" else 1
                    if len(waits) > cap and all(
                        w.get("wait_mode") == "sem-ge-imm" for w in waits
                    ):
                        for w in waits[:-cap]:
                            n[0] += 1
                            nop = {
                                "engine": inst["engine"],
                                "ins": [],
                                "name": f"{inst['name']}-ws{n[0]}",
                                "opcode": "NoOp",
                                "outs": [],
                                "sync_info": {"on_wait": [w], "on_update": []},
                                "text_hint": "wait_split",
                            }
                            if "debug" in inst:
                                nop["debug"] = inst["debug"]
                            out.append(nop)
                        si["on_wait"] = waits[-cap:]
                    out.append(inst)
                obj["instructions"] = out
            for v in obj.values():
                fix(v)
        elif isinstance(obj, list):
            for v in obj:
                fix(v)

    fix(m)
    return json.dumps(m).encode()


def _install_wait_split_patch(bass):
    if getattr(bass.Bass, "_wait_split_patched", False):
        return
    orig = bass.Bass.to_json_bytes

    def to_json_bytes(self, *a, **kw):
        return _split_multi_waits(orig(self, *a, **kw))

    bass.Bass.to_json_bytes = to_json_bytes
    bass.Bass._wait_split_patched = True


def _build():
    import concourse.bass as bass
    import concourse.tile as tile
    from concourse import mybir

    _install_wait_split_patch(bass)
    f32 = mybir.dt.float32
    bf16 = mybir.dt.bfloat16
    f8 = mybir.dt.float8e4
    DR = mybir.MatmulPerfMode.DoubleRow

    nc = bass.Bass(enable_partition_id=False)
    xu = nc.dram_tensor("xu", [P, T, D], bf16, kind="ExternalInput")
    xv = nc.dram_tensor("xv", [P, T, D], bf16, kind="ExternalInput")
    out_h = nc.dram_tensor("out_h", [D, D], f32, kind="ExternalOutput")
    out_p = nc.dram_tensor("out_p", [D, D], f32, kind="ExternalOutput")

    with tile.TileContext(nc) as tc:
        with (
            tc.tile_pool(name="big", bufs=1) as big,
            tc.tile_pool(name="wp", bufs=1) as wp,
            tc.tile_pool(name="psum", bufs=1, space="PSUM") as psum,
            tc.tile_pool(name="ost", bufs=8) as ost,
        ):
            us = big.tile([P, T, D], bf16, name="us", tag="us")
            vs = big.tile([P, T, D], bf16, name="vs", tag="vs")
            rs8 = big.tile([P, T, D], f8, name="rs8", tag="rs8")
            dmy = wp.tile([P, D], bf16, name="dmy", tag="dmy")
            dmf = wp.tile([P, 1], f32, name="dmf", tag="dmf")

            # Warm-up fodder init on the otherwise-idle Pool engine so the
            # PE dummies depend on nothing slow.
            nc.gpsimd.memset(dmy[:], 0.0)
            nc.gpsimd.memset(dmf[:], 0.0)

            ph = [psum.tile([P, D], f32, name=f"ph{m}", tag=f"ph{m}") for m in range(M)]
            pp = [psum.tile([P, D], f32, name=f"pp{m}", tag=f"pp{m}") for m in range(M)]

            # PE warm-up: HAM un-throttles after ~3.4us of sustained matmul
            # activity. Isolated N=512 dummies (~1.2us each cold) bridge
            # until the first input chunk lands (~11us). Dummies go to pp3's
            # bank (its first real start=True matmul discards this).
            for _ in range(3):
                nc.tensor.matmul(pp[3][:], dmy[:, :P], dmy[:], start=True, stop=True)

            # ---- input streaming + Rs8 = fp8(Us + Vs) ----------------
            # xu on the SP ring, xv on the ACT ring (parallel issue); one
            # DVE add per chunk with fp8 output.
            for t0, nt in CHUNKS:
                sl = slice(t0, t0 + nt)
                nc.sync.dma_start(us[:, sl, :], xu[:, sl, :])
                nc.scalar.dma_start(vs[:, sl, :], xv[:, sl, :])
                nc.vector.tensor_add(rs8[:, sl, :], us[:, sl, :], vs[:, sl, :])

            # ACT Copy-table preload: issued after the xv loads so the
            # ~1.3us table load happens off the critical path, well before
            # the first PSUM flush copy needs it.
            nc.scalar.mul(dmf[:], dmf[:], 1.0)

            # ---- Phase A matmuls: tile-pair, bank-major within pair ---
            # H0..3: bf16, 512 cols, two tiles back-to-back per bank.
            # P0, P1: one fp8 DoubleRow matmul per pair (contracts both
            # tiles at once).
            # The first two pairs also run H3 (tiles 0-3): extra PE work
            # while the DMA pipeline fills, so the PE never goes idle (an
            # idle PE resets the HAM ramp window). H3's accumulation group
            # stays open; the H3 pass below finishes tiles 4..31.
            H3_HEAD = 4
            for tp in range(0, T, 2):
                st, sp = (tp == 0), (tp == T - 2)
                nbank = 4 if tp < H3_HEAD else 3
                # Pair 0 runs t0-major: four matmuls of tile-0 work buffer
                # the slightly later tile-1 DMA arrival.
                if tp == 0:
                    order = [(m, t) for t in (0, 1) for m in range(nbank)]
                else:
                    order = [(m, t) for m in range(nbank) for t in (tp, tp + 1)]
                for m, t in order:
                    nc.tensor.matmul(
                        ph[m][:],
                        us[:, t, m * P : (m + 1) * P],
                        vs[:, t, :],
                        start=(t == 0), stop=(t == T - 1),
                    )
                for m in range(2):
                    nc.tensor.matmul(
                        pp[m][:, m * P : D],
                        rs8[:, tp : tp + 2, m * P : (m + 1) * P],
                        rs8[:, tp : tp + 2, m * P : D],
                        start=st, stop=sp,
                        perf_mode=DR,
                    )

            # Flush the five phase-A banks (H0-2, P0, P1) while the H3
            # pass runs on the PE — their ~1.4 MiB of output DMA hides
            # under the remaining compute. Stores on the idle Pool ring so
            # the ACT queue stays clear for pp3's tail flush.
            hstage = ost.tile([P, M, D], f32, name="hstage", tag="hstage")
            out_h3 = out_h.rearrange("(m p) d -> p m d", p=P)
            nc.vector.tensor_copy(hstage[:, 0, :], ph[0][:])
            nc.scalar.copy(hstage[:, 1, :], ph[1][:])
            nc.vector.tensor_copy(hstage[:, 2, :], ph[2][:])
            nc.gpsimd.dma_start(out_h3[:, 0:3, :], hstage[:, 0:3, :])
            for m in range(2):
                c0 = m * P
                o = ost.tile([P, D - c0], f32, name=f"op{m}", tag="ostp")
                if m == 0:
                    nc.vector.tensor_copy(o[:], pp[m][:, c0:D])
                else:
                    nc.scalar.copy(o[:], pp[m][:, c0:D])
                nc.gpsimd.dma_start(out_p[m * P : (m + 1) * P, c0:D], o[:])

            # ---- H3 pass (SBUF-resident, same bank throughout) --------
            for t in range(H3_HEAD, T):
                nc.tensor.matmul(
                    ph[3][:],
                    us[:, t, 3 * P : D],
                    vs[:, t, :],
                    start=(t == 0), stop=(t == T - 1),
                )
            nc.scalar.copy(hstage[:, 3, :], ph[3][:])
            nc.gpsimd.dma_start(out_h3[:, 3:4, :], hstage[:, 3:4, :])

            # ---- Phase B: pp2 (DoubleRow pairs), pp3 (plain fp8) ------
            # pp3 (the last bank) flushes copy+store on one engine (ACT)
            # to keep the tail short.
            for tp in range(0, T, 2):
                nc.tensor.matmul(
                    pp[2][:, 2 * P : D],
                    rs8[:, tp : tp + 2, 2 * P : 3 * P],
                    rs8[:, tp : tp + 2, 2 * P : D],
                    start=(tp == 0), stop=(tp == T - 2),
                    perf_mode=DR,
                )
            o2 = ost.tile([P, D - 2 * P], f32, name="op2", tag="ostp")
            nc.vector.tensor_copy(o2[:], pp[2][:, 2 * P : D])
            nc.gpsimd.dma_start(out_p[2 * P : 3 * P, 2 * P : D], o2[:])

            for t in range(T):
                nc.tensor.matmul(
                    pp[3][:, 3 * P : D],
                    rs8[:, t, 3 * P : D],
                    rs8[:, t, 3 * P : D],
                    start=(t == 0), stop=(t == T - 1),
                )
            o3 = ost.tile([P, D - 3 * P], f32, name="op3", tag="ostp")
            nc.scalar.copy(o3[:], pp[3][:, 3 * P : D])
            nc.scalar.dma_start(out_p[3 * P : D, 3 * P : D], o3[:])

    return nc


def _get_nc():
    if "nc" not in _cache:
        _cache["nc"] = _build()
    return _cache["nc"]


def kernel(input_real, input_imag, weight):
    import ml_dtypes

    from concourse.bass_utils import run_bass_kernel_spmd

    bf16 = ml_dtypes.bfloat16
    input_real = np.asarray(input_real, dtype=np.float32)
    input_imag = np.asarray(input_imag, dtype=np.float32)
    weight = np.asarray(weight, dtype=np.float32)
    a = 0.5 * np.sqrt(weight)  # w >= 0 (uniform fill)

    us_full = (a[:, :, None] * (input_real + input_imag)).astype(bf16)
    vs_full = (a[:, :, None] * (input_real - input_imag)).astype(bf16)

    def pmaj(x):  # [S_LOC, D] -> [P, T, D], s_local = t*P + p
        return np.ascontiguousarray(x.reshape(T, P, D).transpose(1, 0, 2))

    in_maps = []
    for k in range(N_CORES):
        b, h = k // 2, k % 2
        rows = slice(h * S_LOC, (h + 1) * S_LOC)
        in_maps.append({"xu": pmaj(us_full[b, rows]), "xv": pmaj(vs_full[b, rows])})

    res = run_bass_kernel_spmd(
        _get_nc(), in_maps, core_ids=list(range(N_CORES))
    )

    out_r = np.empty((B, D, D), dtype=np.float32)
    out_i = np.empty((B, D, D), dtype=np.float32)
    for b in range(B):
        H = res.results[2 * b]["out_h"].astype(np.float64) + res.results[
            2 * b + 1
        ]["out_h"].astype(np.float64)
        Pu = res.results[2 * b]["out_p"].astype(np.float64) + res.results[
            2 * b + 1
        ]["out_p"].astype(np.float64)
        Pf = np.empty((D, D), dtype=np.float64)
        for m in range(M):
            for n in range(M):
                rm = slice(m * P, (m + 1) * P)
                rn = slice(n * P, (n + 1) * P)
                if m <= n:
                    Pf[rm, rn] = Pu[rm, rn]
                else:
                    Pf[rm, rn] = Pu[rn, rm].T
        Hs = H + H.T
        out_r[b] = (2.0 * Pf - 2.0 * Hs).astype(np.float32)
        out_i[b] = (2.0 * (H - H.T)).astype(np.float32)
    return out_r, out_i
